# revision 80
# baseline (speedup 1.0000x reference)
"""Trainium2 Bass kernel for one GPT-style transformer block.

Problem: x[8,1024,1024]; per-core = one batch element (data-parallel over 8
NeuronCores).  Per core:
    h1 = LN(x); qkv = h1@Wqkv+b; causal MHA (16 heads, d=64);
    r1 = x + attn@Wproj+b; h2 = LN(r1); out = r1 + relu(h2@W1+b1)@W2+b2

Measured on TRN2: 598 us/core (v1 baseline: 861 us), rel err 3.8e-4.

Design (v1 trace showed PE matmul busy 81% but attention matmuls ran at
~2.6x ideal because the in-order PE queue stalled on the exp dependency
and never reached the 2.4GHz p-state):
  - LN gains/biases are folded into the downstream weights on the host
    (LN(x)@W = ((x-mu)*inv)@(g.*W) + (b@W + bias)); the kernel computes
    only (x-mu)*inv.
  - LN1 runs token-major during the x load: per-token mean/meansq via DVE
    free-axis reduces, apply as one ACT op/tile (per-partition scale+bias);
    x and h1 are then PE-transposed to feature-major (evictions batched 4
    chunks per wide ACT copy into big [P,8,T] tiles).
  - Linears: matmul(lhsT=W[K,M], rhs=act[K,N]) in f32r, N=512.
  - Attention internals (q,k,v,vaug,P) in bf16.  A head-block's two
    parities share wide [128,1024] score tiles (cols 0:512 parity0) so exp
    is one wide ACT instruction; exp'd scores land in SBUF.  Emission
    interleaves ~3 independent filler matmul units (next head-block's QKV
    split into 2-mm units, lagged PV accumulations) after each ST tile, so
    the in-order PE queue never waits on the Activation engine.
  - Softmax denominators via a ones column in vaug (row 64 of PV psum);
    reciprocal on partition-base-0 rows; per-(hb,qi) rank-1 broadcast
    matmul deferred into the next PE-busy stretch.
  - proj runs t-outer (Wproj re-DMAed per t) with LN2 stats accumulated
    in-stream; LN2 apply overlaps the opposite t's proj/FFN1.
  - FFN in 4 d_ff quarters; FFN2 partials accumulate into xf via DVE adds;
    out-transposes + per-m column DMA are emitted inside the last quarter.
  - PSUM: wide 2x4KB (ST/stats/broadcast) + linear 2x2KB (linears,
    transposes, small broadcasts) + PV 2x2KB = 16KB/partition exactly.
"""

import math
import sys

import numpy as np

sys.path.insert(0, "/opt/trn_rl_repo")

from contextlib import ExitStack

import concourse.bass as bass
import concourse.mybir as mybir
import concourse.tile as tile
from concourse import bacc
from concourse.bass import ts
from concourse.masks import make_identity

F32 = mybir.dt.float32
F32R = mybir.dt.float32r
BF16 = mybir.dt.bfloat16
AF = mybir.ActivationFunctionType
ALU = mybir.AluOpType
AX = mybir.AxisListType

B, T, C, H = 8, 1024, 1024, 16
D = C // H
FF = 4 * C
P = 128
NCH = C // P          # 8 feature chunks
NT = T // P           # 8 token chunks of 128
NQ = T // 512         # 2 token chunks of 512
SCALE = 1.0 / math.sqrt(3 * C // H)
EPS = 1e-5


def _build():
    nc = bacc.Bacc("TRN2", target_bir_lowering=False, debug=False)

    x_d = nc.dram_tensor("x", [T, C], F32R, kind="ExternalInput").ap()
    Wqkv_d = nc.dram_tensor("Wqkv", [C, 3 * C], F32R, kind="ExternalInput").ap()
    bqkv_d = nc.dram_tensor("bqkv", [3 * C], F32, kind="ExternalInput").ap()
    Wproj_d = nc.dram_tensor("Wproj", [C, C], F32R, kind="ExternalInput").ap()
    bproj_d = nc.dram_tensor("bproj", [C], F32, kind="ExternalInput").ap()
    W1_d = nc.dram_tensor("W1", [C, FF], F32R, kind="ExternalInput").ap()
    b1_d = nc.dram_tensor("b1", [FF], F32, kind="ExternalInput").ap()
    W2_d = nc.dram_tensor("W2", [FF, C], F32R, kind="ExternalInput").ap()
    b2_d = nc.dram_tensor("b2", [C], F32, kind="ExternalInput").ap()
    out_d = nc.dram_tensor("out", [T, C], F32, kind="ExternalOutput").ap()

    Wqkv_r = Wqkv_d.rearrange("(j p) m -> p j m", p=P)     # [128, 8, 3072]
    Wproj_r = Wproj_d.rearrange("(j p) m -> p j m", p=P)   # [128, 8, 1024]
    W1_r = W1_d.rearrange("(j p) m -> p j m", p=P)         # [128, 8, 4096]
    W2_r = W2_d.rearrange("(j p) m -> p j m", p=P)         # [128, 32, 1024]

    with nc.allow_low_precision(reason="fp32r/bf16 matmul inputs (fp32 accum)"), \
         tile.TileContext(nc) as tc, ExitStack() as ctx:
        const = ctx.enter_context(tc.tile_pool(name="const", bufs=1))
        xpool = ctx.enter_context(tc.tile_pool(name="xpool", bufs=1))
        hpool = ctx.enter_context(tc.tile_pool(name="hpool", bufs=1))
        spool = ctx.enter_context(tc.tile_pool(name="spool", bufs=2))
        wpool = ctx.enter_context(tc.tile_pool(name="wpool", bufs=2))
        qkv_ctx = ExitStack()   # closed just before the FFN block
        qkvp = qkv_ctx.enter_context(tc.tile_pool(name="qkvp", bufs=1))
        # PSUM: wide 2x4KB + lin 2x2KB + pv 2x2KB = 16KB/partition
        ps_st = ctx.enter_context(tc.tile_pool(name="ps_st", bufs=2, space="PSUM"))
        ps_lin = ctx.enter_context(tc.tile_pool(name="ps_lin", bufs=2, space="PSUM"))
        ps_pv = ctx.enter_context(tc.tile_pool(name="ps_pv", bufs=2, space="PSUM"))

        ident_r = const.tile([P, P], F32R)
        ident_b = const.tile([P, P], BF16)
        identf = const.tile([P, P], F32)
        ones_f = const.tile([P, 1], F32)
        nc.vector.memset(ones_f[:], 1.0)
        ones8 = const.tile([P, NT], F32)
        nc.vector.memset(ones8[:], 1.0)
        ones_col = const.tile([P, 1], F32R)
        nc.scalar.activation(ones_col[:], ones_f[:], AF.Copy)
        ones_row = const.tile([1, P], F32R)
        eps_col = const.tile([P, 1], F32)
        nc.vector.memset(eps_col[:], EPS)
        zero_col = const.tile([P, 1], F32)
        nc.vector.memset(zero_col[:], 0.0)

        masks = []
        with tc.tile_pool(name="mbuild", bufs=2) as mbp:
            make_identity(nc, identf[:])
            nc.scalar.activation(ident_r[:], identf[:], AF.Copy)
            nc.scalar.activation(ident_b[:], identf[:], AF.Copy)
            ones_rowf = mbp.tile([1, P], F32, tag="orow", name="ones_rowf")
            nc.vector.memset(ones_rowf[:], 1.0)
            nc.scalar.activation(ones_row[:], ones_rowf[:], AF.Copy)
            # causal mask per diagonal d: mask_d[r, c] = 1 if c - r >= d*128
            for di in range(4):
                mf = mbp.tile([P, 512], F32, tag="mf", name=f"mf{di}")
                nc.gpsimd.memset(mf[:], 1.0)
                nc.gpsimd.affine_select(
                    out=mf[:], in_=mf[:], pattern=[[1, 512]],
                    base=-di * P, channel_multiplier=-1,
                    compare_op=ALU.is_ge, fill=0.0)
                mk = const.tile([P, 512], BF16, tag=f"mask{di}", name=f"mask{di}")
                nc.scalar.activation(mk[:], mf[:], AF.Copy)
                masks.append(mk)

        # bias columns (col m = vec[m*128:(m+1)*128]): DMA row-major (few
        # big descriptors), then one PE transpose per vector
        bqkv_t = const.tile([P, 3 * NCH], F32)
        bproj_t = const.tile([P, NCH], F32)
        b1_t = const.tile([P, FF // P], F32)
        b2_t = const.tile([P, NCH], F32)
        with tc.tile_pool(name="bload", bufs=4) as blp:
            for src_d, dst, nr in ((bqkv_d, bqkv_t, 3 * NCH),
                                   (bproj_d, bproj_t, NCH),
                                   (b1_d, b1_t, FF // P),
                                   (b2_d, b2_t, NCH)):
                tmp = blp.tile([nr, P], F32, tag="btmp", name=f"btmp{nr}")
                nc.sync.dma_start(tmp[:], src_d.rearrange("(m p) -> m p", p=P))
                pst = ps_lin.tile([P, 512], F32, tag="lin", name=f"btr{nr}")
                nc.tensor.transpose(pst[:, 0:nr], tmp[:], identf[0:nr, 0:nr])
                nc.scalar.activation(dst[:], pst[:, 0:nr], AF.Copy)

        # persistent feature-major big tiles: [P, chunk, T]
        xf = xpool.tile([P, NCH, T], F32R, tag="x", name="xf")
        h1f = hpool.tile([P, NCH, T], F32R, tag="h", name="h1f")

        def make_qkv(hb):
            """Alloc q/k/v + weight DMAs; emission via generator units."""
            q = qkvp.tile([P, T], BF16, tag="qk", bufs=4, name=f"q{hb}")
            k = qkvp.tile([P, T], BF16, tag="qk", bufs=4, name=f"k{hb}")
            v = qkvp.tile([P, T], BF16, tag="v", bufs=2, name=f"v{hb}")
            wts = []
            for li, lname, dst in ((2, "v", v), (1, "k", k), (0, "q", q)):
                w = wpool.tile([P, NCH, P], F32R, tag="wqkv", bufs=3,
                               name=f"w{lname}{hb}")
                nc.sync.dma_start(w[:], Wqkv_r[:, :, ts(li * NCH + hb, P)])
                wts.append((li, lname, dst, w))

            def gen():
                for t in range(NQ):
                    for li, lname, dst, w in wts:
                        mcol = li * NCH + hb
                        ps = ps_lin.tile([P, 512], F32, tag="lin",
                                         name=f"{lname}{hb}ps{t}")
                        for j in range(NCH):
                            nc.tensor.matmul(
                                ps[:], w[:, j, :], h1f[:, j, ts(t, 512)],
                                start=(j == 0), stop=(j == NCH - 1))
                            if j % 2 == 1 and j < NCH - 1:
                                yield
                        nc.vector.tensor_scalar_add(
                            dst[:, ts(t, 512)], ps[:],
                            bqkv_t[:, mcol:mcol + 1])
                        yield
            return q, k, v, gen()

        # ---------------- load x; token-major LN1; transpose both ------------
        with tc.tile_pool(name="xload", bufs=8) as xlp:
            xtm = [xlp.tile([P, C], F32R, tag="xtm", bufs=4, name=f"xtm{i}")
                   for i in range(NT)]
            htm = [xlp.tile([P, C], F32R, tag="htm", bufs=4, name=f"htm{i}")
                   for i in range(NT)]
            def load_x(i):
                # split across DMA queues (one queue is only ~22GB/s);
                # first two tiles 8-way for the earliest possible arrival
                nch_ = 8 if i < 2 else 4
                w = C // nch_
                for ch in range(nch_):
                    nc.sync.dma_start(xtm[i][:, ts(ch, w)],
                                      x_d[ts(i, P), ts(ch, w)])

            for i in range(4):
                load_x(i)
            qkv0 = make_qkv(0)

            def ln1_tm(i):
                scr = xlp.tile([P, C], F32, tag="scr", bufs=1, name=f"scr{i}")
                cols = {}
                for nm in ("s", "q", "mu", "var", "musq", "sd", "inv", "c0"):
                    cols[nm] = spool.tile([P, 1], F32, tag="lncol", bufs=24,
                                          name=f"{nm}{i}")
                nc.vector.tensor_reduce(cols["s"][:], xtm[i][:], AX.X, ALU.add)
                nc.scalar.activation(scr[:], xtm[i][:], AF.Square,
                                     accum_out=cols["q"][:])
                nc.scalar.mul(cols["mu"][:], cols["s"][:], 1.0 / C)
                nc.scalar.mul(cols["var"][:], cols["q"][:], 1.0 / C)
                nc.vector.tensor_mul(cols["musq"][:], cols["mu"][:],
                                     cols["mu"][:])
                nc.vector.tensor_sub(cols["var"][:], cols["var"][:],
                                     cols["musq"][:])
                nc.scalar.activation(cols["sd"][:], cols["var"][:], AF.Sqrt,
                                     bias=eps_col[:])
                nc.vector.reciprocal(cols["inv"][:], cols["sd"][:])
                nc.vector.tensor_mul(cols["c0"][:], cols["mu"][:],
                                     cols["inv"][:])
                nc.scalar.mul(cols["c0"][:], cols["c0"][:], -1.0)
                nc.scalar.activation(htm[i][:], xtm[i][:], AF.Identity,
                                     bias=cols["c0"][:], scale=cols["inv"][:])

            def tr_to_fm(dstf, src_i, i, nm, on_dve):
                for g in range(2):
                    pst = ps_lin.tile([P, 512], F32R, tag="lin",
                                      name=f"{nm}{i}_{g}")
                    for mi in range(4):
                        nc.tensor.transpose(pst[:, ts(mi, P)],
                                            src_i[:, ts(4 * g + mi, P)],
                                            ident_r[:])
                    dst = dstf[:, 4 * g:4 * g + 4, ts(i, P)]
                    src = pst[:].rearrange("p (mi f) -> p mi f", mi=4)
                    if on_dve:
                        nc.vector.tensor_copy(dst, src)
                    else:
                        nc.scalar.activation(dst, src, AF.Copy)

            for i in range(NT):
                tr_to_fm(xf, xtm[i][:], i, "xtr", True)
                ln1_tm(i)
                if i + 4 < NT:
                    load_x(i + 4)
                if i > 0:
                    tr_to_fm(h1f, htm[i - 1][:], i - 1, "htr", False)
                if i == 4:
                    # h1 token-rows 0:512 are complete: overlap QKV(0) t=0
                    for _ in range(12):
                        next(qkv0[3], None)
            tr_to_fm(h1f, htm[NT - 1][:], NT - 1, "htr", False)

        # ---------------- attention: pipelined per head-block ----------------
        with tc.tile_pool(name="ptp", bufs=1) as ptp, \
             tc.tile_pool(name="ypool", bufs=1) as ypool:

            yf = ypool.tile([P, NCH, T], F32R, tag="y", name="yf")

            def v_transpose(hb, v):
                va = qkvp.tile([P, NT, 130], BF16, tag="vaug", bufs=2,
                               name=f"va{hb}")
                for g in range(2):
                    pst = ps_lin.tile([P, 512], BF16, tag="lin",
                                      name=f"vtr{hb}_{g}")
                    for ki in range(4):
                        nc.tensor.transpose(pst[:, ts(ki, P)],
                                            v[:, ts(4 * g + ki, P)],
                                            ident_b[:])
                    dst = va[:, 4 * g:4 * g + 4, :].rearrange(
                        "p k (h c) -> p k h c", h=2)[:, :, :, 0:64]
                    src = pst[:].rearrange("p (k h c) -> p k h c", k=4, h=2)
                    nc.vector.tensor_copy(dst, src)
                nc.vector.tensor_copy(
                    va[:, :, 64:65].rearrange("p k o -> p (k o)"), ones8[:])
                nc.vector.tensor_copy(
                    va[:, :, 129:130].rearrange("p k o -> p (k o)"), ones8[:])
                return va

            def emit_st(hb, qi, ki, q, k):
                """One wide ST tile + exp (+mask); returns the P tile."""
                st = ps_st.tile([P, T], F32, tag="st", name=f"st{hb}_{qi}_{ki}")
                for p_ in range(2):
                    nc.tensor.matmul(
                        st[:, ts(p_, 512)],
                        k[p_ * 64:(p_ + 1) * 64, ts(ki, P)],
                        q[p_ * 64:(p_ + 1) * 64, ts(qi, 512)],
                        start=True, stop=True)
                pt = ptp.tile([P, T], BF16, tag="pt", bufs=12,
                              name=f"pt{hb}_{qi}_{ki}")
                nc.scalar.activation(pt[:], st[:], AF.Exp,
                                     bias=zero_col[:], scale=SCALE)
                d = ki - 4 * qi
                if d >= 0:
                    for p_ in range(2):
                        nc.vector.tensor_mul(pt[:, ts(p_, 512)],
                                             pt[:, ts(p_, 512)], masks[d][:])
                return pt

            def make_pv(hb, qi, va, pts, out):
                """Generator: PV accumulation in 2-mm units, then dn chain."""
                def gen():
                    pvs = []
                    kmax = 4 * qi + 3
                    for p_ in range(2):
                        pv = ps_pv.tile([P, 512], F32, tag="pv",
                                        name=f"pv{hb}_{qi}_{p_}")
                        for ki in range(kmax + 1):
                            nc.tensor.matmul(
                                pv[0:65, :],
                                va[:, ki, p_ * 65:(p_ + 1) * 65],
                                pts[ki][:, ts(p_, 512)],
                                start=(ki == 0), stop=(ki == kmax))
                            if ki % 2 == 1:
                                yield
                        pvs.append(pv)
                    dnrow = qkvp.tile([1, T], F32, tag="dnrow", bufs=3,
                                      name=f"dr{hb}_{qi}")
                    for p_ in range(2):
                        nc.scalar.activation(dnrow[0:1, ts(p_, 512)],
                                             pvs[p_][64:65, :], AF.Copy)
                    dn = qkvp.tile([1, T], F32, tag="dnrow", bufs=3,
                                   name=f"dn{hb}_{qi}")
                    nc.vector.reciprocal_approx_fast(dn[:], dnrow[:])
                    dnr = qkvp.tile([1, T], F32R, tag="dnrow", bufs=3,
                                    name=f"dq{hb}_{qi}")
                    nc.vector.tensor_copy(dnr[:], dn[:])
                    out.extend([pvs, dnr])
                return gen()

            def dn_bcast_finish(hb, qi, pvs, dnr, use_lin=False):
                """Broadcast 1/den over 64 partitions; write y (f32r)."""
                dnb = qkvp.tile([64, T], F32R, tag="dnb", bufs=1,
                                name=f"dnbs{hb}_{qi}")
                if use_lin:
                    for p_ in range(2):
                        bps = ps_lin.tile([P, 512], F32, tag="lin",
                                          name=f"dnb{hb}_{qi}_{p_}")
                        nc.tensor.matmul(bps[0:64, :], ones_row[:, 0:64],
                                         dnr[0:1, ts(p_, 512)],
                                         start=True, stop=True)
                        nc.scalar.activation(dnb[:, ts(p_, 512)],
                                             bps[0:64, :], AF.Copy)
                else:
                    bps = ps_st.tile([P, T], F32, tag="st",
                                     name=f"dnb{hb}_{qi}")
                    for p_ in range(2):
                        nc.tensor.matmul(bps[0:64, ts(p_, 512)],
                                         ones_row[:, 0:64],
                                         dnr[0:1, ts(p_, 512)],
                                         start=True, stop=True)
                    nc.vector.tensor_copy(dnb[:], bps[0:64, :])
                for p_ in range(2):
                    nc.vector.tensor_mul(
                        yf[p_ * 64:(p_ + 1) * 64, hb, ts(qi, 512)],
                        pvs[p_][0:64, :], dnb[:, ts(p_, 512)])

            def pump(g, n=None):
                if g is None:
                    return True
                try:
                    if n is None:
                        while True:
                            next(g)
                    else:
                        for _ in range(n):
                            next(g)
                except StopIteration:
                    return True
                return False

            # --- pipelined head-block loop ---
            q, k, v, gq = qkv0
            pump(gq)                       # finish QKV(0) (t=0 ran in load)
            gq = None
            prev1 = None                   # (hb, va, pts1) -> PV in next iter
            pend0 = None                   # (hb, pvs0, dnr0) -> bcast next iter
            for hb in range(NCH):
                va = v_transpose(hb, v)
                if hb < NCH - 1:
                    qn, kn, vn, gq = make_qkv(hb + 1)
                else:
                    qn = kn = vn = gq = None
                res1 = []
                gpv1 = None
                pv1_done = prev1 is None
                if prev1 is not None:
                    phb, pva, ppts1 = prev1
                    gpv1 = make_pv(phb, 1, pva, ppts1, res1)
                res0 = []
                gpv0 = None
                pts0, pts1 = [], []
                st_items = [(0, ki) for ki in range(4)] + \
                           [(1, ki) for ki in range(NT)]
                for idx, (qi, ki) in enumerate(st_items):
                    pt = emit_st(hb, qi, ki, q, k)
                    (pts0 if qi == 0 else pts1).append(pt)
                    if idx == 0 and pend0 is not None:
                        dn_bcast_finish(pend0[0], 0, pend0[1], pend0[2])
                        pend0 = None
                    if gpv1 is None and not pv1_done:
                        dn_bcast_finish(phb, 1, res1[0], res1[1])
                        pv1_done = True
                    if idx == 7:
                        if gpv1 is not None:
                            pump(gpv1)
                            gpv1 = None
                        if not pv1_done:
                            dn_bcast_finish(phb, 1, res1[0], res1[1])
                            pv1_done = True
                        gpv0 = make_pv(hb, 0, va, pts0, res0)
                    for _ in range(3):
                        if gpv1 is not None:
                            if pump(gpv1, 1):
                                gpv1 = None
                        elif idx >= 8 and gpv0 is not None:
                            if pump(gpv0, 1):
                                gpv0 = None
                        elif gq is not None:
                            if pump(gq, 1):
                                gq = None
                pump(gpv0)
                pump(gq)
                gq = None
                pend0 = (hb, res0[0], res0[1])
                prev1 = (hb, va, pts1)
                q, k, v = qn, kn, vn
                if hb == NCH - 2:
                    # pre-add bproj during iter-6 ACT slack, off the
                    # post-attention critical path
                    for m in range(NCH):
                        nc.scalar.activation(xf[:, m, :], xf[:, m, :],
                                             AF.Identity,
                                             bias=bproj_t[:, m:m + 1],
                                             scale=1.0)

            # --- leftovers + proj + LN2 ---
            dn_bcast_finish(pend0[0], 0, pend0[1], pend0[2])
            phb, pva, ppts1 = prev1
            res1 = []
            pump(make_pv(phb, 1, pva, ppts1, res1))

            ln2_stats = [ps_st.tile([1, T], F32, tag="st", name=f"ln2_st{t}")
                         for t in range(NQ)]

            def ln2_sq(m, t):
                sq = spool.tile([P, 512], F32R, tag="sq", bufs=2,
                                name=f"ln2sq{m}_{t}")
                nc.gpsimd.tensor_mul(sq[:], xf[:, m, ts(t, 512)],
                                     xf[:, m, ts(t, 512)])
                return sq

            def ln2_stat_mms(m, t, sq):
                nc.tensor.matmul(ln2_stats[t][0:1, 0:512],
                                 ones_col[:], xf[:, m, ts(t, 512)],
                                 start=(m == 0), stop=(m == NCH - 1))
                nc.tensor.matmul(ln2_stats[t][0:1, 512:1024],
                                 ones_col[:], sq[:],
                                 start=(m == 0), stop=(m == NCH - 1))

            def ln2_finalize(t):
                rows = spool.tile([1, T], F32R, tag="lnrows", bufs=1,
                                  name=f"ln2rows{t}")
                mu = spool.tile([1, 512], F32R, tag="lnsm", bufs=4,
                                name=f"ln2mu{t}")
                var = spool.tile([1, 512], F32, tag="lnsm", bufs=4,
                                 name=f"ln2var{t}")
                nc.scalar.mul(mu[:], ln2_stats[t][0:1, 0:512], 1.0 / C)
                nc.scalar.mul(var[:], ln2_stats[t][0:1, 512:1024], 1.0 / C)
                musq = spool.tile([1, 512], F32, tag="lnsm", bufs=4,
                                  name=f"ln2musq{t}")
                nc.vector.tensor_mul(musq[:], mu[:], mu[:])
                nc.vector.tensor_sub(var[:], var[:], musq[:])
                sd = spool.tile([1, 512], F32, tag="lnsm", bufs=4,
                                name=f"ln2sd{t}")
                nc.scalar.activation(sd[:], var[:], AF.Sqrt,
                                     bias=eps_col[0:1, :])
                nc.vector.reciprocal(rows[0:1, 0:512], sd[:])
                nc.vector.tensor_mul(rows[0:1, 512:1024], mu[:],
                                     rows[0:1, 0:512])
                nc.scalar.mul(rows[0:1, 512:1024], rows[0:1, 512:1024], -1.0)
                bps = ps_st.tile([P, T], F32, tag="st", name=f"ln2bps{t}")
                for half in range(2):
                    nc.tensor.matmul(bps[:, ts(half, 512)], ones_row[:],
                                     rows[0:1, ts(half, 512)],
                                     start=True, stop=True)
                bc = spool.tile([P, T], F32R, tag="lnbc", bufs=2,
                                name=f"ln2bc{t}")
                nc.scalar.activation(bc[:], bps[:], AF.Copy)
                return bc

            h2f = hpool.tile([P, NCH, T], F32R, tag="h", name="h2f")

            def ln2_apply(c, t, bc):
                # stage via PSUM: 3-SBUF-operand DVE ops run at 1/3 rate
                ps = ps_pv.tile([P, 512], F32, tag="pv", name=f"ap{c}_{t}")
                nc.vector.tensor_mul(ps[:], xf[:, c, ts(t, 512)],
                                     bc[:, 0:512])
                nc.vector.tensor_add(h2f[:, c, ts(t, 512)], ps[:],
                                     bc[:, 512:1024])

            def proj_pass(t, first, do_stats):
                ws = {}

                def load(m):
                    if m < NCH:
                        w = wpool.tile([P, NCH, P], F32R, tag="wqkv", bufs=3,
                                       name=f"wproj{m}_{t}")
                        nc.sync.dma_start(w[:], Wproj_r[:, :, ts(m, P)])
                        ws[m] = w

                load(0)
                load(1)
                sqs = {}
                for m in range(NCH):
                    load(m + 2)
                    ps = ps_lin.tile([P, 512], F32, tag="lin",
                                     name=f"proj_ps{m}_{t}")
                    for j in range(NCH):
                        nc.tensor.matmul(ps[:], ws[m][:, j, :],
                                         yf[:, j, ts(t, 512)],
                                         start=(j == 0), stop=(j == NCH - 1))
                    if first and m == 0:
                        # finish (7, q1) while proj keeps the PE busy
                        dn_bcast_finish(phb, 1, res1[0], res1[1], use_lin=True)
                    nc.vector.tensor_add(xf[:, m, ts(t, 512)],
                                         xf[:, m, ts(t, 512)], ps[:])
                    # squares on Pool; stat matmuls lag one group so the PE
                    # never waits on the Pool queue
                    sqs[m] = ln2_sq(m, t)
                    if m > 0:
                        ln2_stat_mms(m - 1, t, sqs[m - 1])
                ln2_stat_mms(NCH - 1, t, sqs[NCH - 1])

            proj_pass(0, True, True)
            bc0 = ln2_finalize(0)
            for c in range(NCH):
                ln2_apply(c, 0, bc0)      # DVE; overlaps proj t=1
            proj_pass(1, False, True)
            bc1 = ln2_finalize(1)
            for c in range(NCH):
                ln2_apply(c, 1, bc1)      # DVE; overlaps FFN1 t=0
            for m in range(NCH):          # pre-add b2 (after stats read)
                nc.scalar.activation(xf[:, m, :], xf[:, m, :], AF.Identity,
                                     bias=b2_t[:, m:m + 1], scale=1.0)

        # ---------------- FFN (4 d_ff quarters) + residual + out -------------
        qkv_ctx.close()
        with tc.tile_pool(name="ffnp", bufs=1) as ffnp:

            def ffn1_group(mg, t, a, wt):
                ps = ps_lin.tile([P, 512], F32, tag="lin",
                                 name=f"f1ps{mg}_{t}")
                for j in range(NCH):
                    nc.tensor.matmul(ps[:], wt[:, j, :],
                                     h2f[:, j, ts(t, 512)],
                                     start=(j == 0), stop=(j == NCH - 1))
                nc.scalar.activation(a[:, ts(t, 512)], ps[:], AF.Relu,
                                     bias=b1_t[:, mg:mg + 1], scale=1.0)

            for qtr in range(4):
                a1, wts = [], []
                for mm_ in range(8):
                    mg = qtr * 8 + mm_
                    a = ffnp.tile([P, T], F32R, tag="a1", bufs=8,
                                  name=f"a1_{mg}")
                    wt = ffnp.tile([P, NCH, P], F32R, tag="w1", bufs=8,
                                   name=f"w1_{mg}")
                    nc.sync.dma_start(wt[:], W1_r[:, :, ts(mg, P)])
                    a1.append(a)
                    wts.append(wt)
                for mm_ in range(8):
                    ffn1_group(qtr * 8 + mm_, 0, a1[mm_], wts[mm_])
                for mm_ in range(8):
                    ffn1_group(qtr * 8 + mm_, 1, a1[mm_], wts[mm_])

                w2s = {}

                def load2(m, qtr=qtr):
                    if m < NCH:
                        w2t = ffnp.tile([P, 8, P], F32R, tag="w2", bufs=3,
                                        name=f"w2_{qtr}_{m}")
                        nc.sync.dma_start(
                            w2t[:], W2_r[:, qtr * 8:(qtr + 1) * 8, ts(m, P)])
                        w2s[m] = w2t

                load2(0)
                load2(1)
                for m in range(NCH):
                    load2(m + 2)
                    for t in range(NQ):
                        ps = ps_lin.tile([P, 512], F32, tag="lin",
                                         name=f"f2ps{qtr}_{m}_{t}")
                        for j in range(8):
                            nc.tensor.matmul(ps[:], w2s[m][:, j, :],
                                             a1[j][:, ts(t, 512)],
                                             start=(j == 0), stop=(j == 7))
                        nc.vector.tensor_add(xf[:, m, ts(t, 512)],
                                             xf[:, m, ts(t, 512)], ps[:])
                    if qtr == 3:
                        # xf[:, m] final: transpose + column-DMA out
                        om = ffnp.tile([P, T], F32, tag="om", bufs=2,
                                       name=f"om{m}")
                        for g in range(2):
                            pst = ps_lin.tile([P, 512], F32R, tag="lin",
                                              name=f"otr{m}_{g}")
                            for ii in range(4):
                                nc.tensor.transpose(
                                    pst[:, ts(ii, P)],
                                    xf[:, m, ts(4 * g + ii, P)], ident_r[:])
                            nc.scalar.activation(om[:, ts(g, 512)], pst[:],
                                                 AF.Copy)
                        dst = out_d[:, ts(m, P)].rearrange(
                            "(i p) f -> p i f", p=P)
                        src = om[:].rearrange("p (i f) -> p i f", i=NT)
                        nc.sync.dma_start(dst, src)

    nc.compile()
    return nc


_NC_CACHE = {}


def _get_nc():
    if "nc" not in _NC_CACHE:
        _NC_CACHE["nc"] = _build()
    return _NC_CACHE["nc"]


def _fold_inputs(inputs):
    """Fold LN gains/biases into the downstream weights (exact algebra)."""
    f = lambda kk: np.asarray(inputs[kk], dtype=np.float32)
    Wqkv, bqkv = f("Wqkv"), f("bqkv")
    W1, b1 = f("W1"), f("b1")
    ln1_g, ln1_b = f("ln1_g"), f("ln1_b")
    ln2_g, ln2_b = f("ln2_g"), f("ln2_b")
    return {
        "Wqkv": np.ascontiguousarray(ln1_g[:, None] * Wqkv),
        "bqkv": np.ascontiguousarray(bqkv + ln1_b @ Wqkv),
        "Wproj": np.ascontiguousarray(f("Wproj")),
        "bproj": np.ascontiguousarray(f("bproj")),
        "W1": np.ascontiguousarray(ln2_g[:, None] * W1),
        "b1": np.ascontiguousarray(b1 + ln2_b @ W1),
        "W2": np.ascontiguousarray(f("W2")),
        "b2": np.ascontiguousarray(f("b2")),
    }


def kernel(**inputs):
    from concourse.bass_utils import run_bass_kernel_spmd

    nc = _get_nc()
    shared = _fold_inputs(inputs)
    x = np.asarray(inputs["x"], dtype=np.float32)
    in_maps = [dict(shared, x=np.ascontiguousarray(x[i])) for i in range(B)]
    res = run_bass_kernel_spmd(nc, in_maps, core_ids=list(range(B)))
    out = np.stack([res.results[i]["out"] for i in range(B)], axis=0)
    return out.astype(np.float32)
